# revision 44
# baseline (speedup 1.0000x reference)
"""Multi-headed attention TRN2 Bass kernel (DMA-byte-minimized).

Problem: B=2, S=2048, D=1024, H=16 heads (dh=64), fp32 ref, bool mask.

Sharding (8 cores): data-parallel over B (2) x tensor-parallel over heads
(4 heads / 256 features per core). Host sums the 4 partials per batch
element (the TP all-reduce) and adds the bias.

The axon-tunneled cores deliver only ~15-50 GB/s of HBM DMA bandwidth
(measured), so the kernel is DMA-bound: every streamed byte is minimized.
  - x, Wq/Wk/Wv/Wo, output partials: fp16 (matmuls run fp16 into fp32
    PSUM at full PE rate).
  - mask: bit-packed int16 (16 q-positions per word, bit-plane-major).
    On-chip expansion: one chained (lsr, and) tensor_scalar per bit-plane
    to {0,1}, one mult(-1) to {0,0xFFFF}; applied to the fp16 exp weights
    with bitwise AND (exact).
Per-core HBM traffic: x 4MB + W 2MB + mask 0.5MB + out 4MB = 10.5MB
(baseline was 28MB). Traffic is spread across all three DMA paths
(sync HWDGE, scalar HWDGE, gpsimd SWDGE) with inputs balanced against
the output stream, and wo's DMA is issued after phase 1 so the x
stream owns the ring heads.

Pipeline structure (from TimelineSim slice-gap analysis):
  - mask expansion runs in phase 1 where DVE is idle (expanding at qc
    boundaries stalled ACT ~5us per chunk);
  - softmax-normalize broadcasts write into the dead denominator halves
    of the ctx PSUM banks (allocating them from the scores tag stalled
    the next chunk's scores);
  - phase 4 for chunk qc-1 is interleaved one ft-tile per 4 kt into
    chunk qc's attention loop (a serial phase-4 block between chunks
    stalled ACT ~8us per boundary); the PE rides its ACT-bound slack.

Per-core compute (all layouts feature-major / transposed):
  phase 1: qT/kT pair tiles [128 feat, S] and v tiles [128 s, 2x(va|1|vb)]
           via fp16 matmuls from xT [D, S]; q/k evictions on ACT (idle in
           phase 1), v evictions on DVE.
  phase 2: per (qc, pair, kt): scores for the head pair via row-packed
           K=64 matmuls into one 2-bank PSUM tile, ONE fused exp over
           [128, 1024] (ACT), multiplicative keep mask (DVE, fp16 4x),
           ctx+denominator accumulation (fp16 matmuls, [v|1] packing).
  phase 3: softmax normalization by reciprocal-multiply at PSUM eviction.
  phase 4: folded into the qc loop: partial outT tile [D, 512] per qc is
           matmul'd, staged fp16, and DMA'd out while later qc's attention
           still runs.

No max-subtraction in softmax: scores are ~N(0,1), exp in fp32->fp16 is
exact enough (rel err ~1e-3 end to end vs the fp32 reference; gate 2e-2).
"""

import math
from contextlib import ExitStack

import numpy as np

import concourse.mybir as mybir
import concourse.tile as tile
from concourse import bacc
from concourse.bass_utils import run_bass_kernel_spmd

B, S, D, H = 2, 2048, 1024, 16
DH = D // H                 # 64
NCORES = 8
GROUPS = NCORES // B        # 4 head-groups per batch element
FPC = D // GROUPS           # 256 features (4 heads) per core
P = 128
SC = 512                    # q/s chunk (free dim of most matmuls)
NQC = S // SC               # 4
NKT = S // P                # 16 k-position tiles
NDT = D // P                # 8 contraction tiles over D

F32 = mybir.dt.float32
F16 = mybir.dt.float16
I16 = mybir.dt.int16
W16 = 32                    # packed-mask words per (kt, bitplane)

EXP = mybir.ActivationFunctionType.Exp
MULT = mybir.AluOpType.mult
AND = mybir.AluOpType.bitwise_and
LSR = mybir.AluOpType.logical_shift_right

# dev bisection knob: "full", "dma", "noattn"
VARIANT = "full"


def _emit(ctx: ExitStack, tc: tile.TileContext, xT, wqT, wkT, wvT, woT, keepT, outT):
    nc = tc.nc

    const = ctx.enter_context(tc.tile_pool(name="const", bufs=1))
    sb = ctx.enter_context(tc.tile_pool(name="sb", bufs=1))
    # all 4 x chunks may be in flight at once: with bufs=2 the later
    # chunks' DMAs gate on earlier projections draining a buffer, which
    # needlessly serializes the scarce DMA stream
    xtp = ctx.enter_context(tc.tile_pool(name="xtp", bufs=4))
    keepp = ctx.enter_context(tc.tile_pool(name="keepp", bufs=4))
    wp = ctx.enter_context(tc.tile_pool(name="wp", bufs=4))
    stg = ctx.enter_context(tc.tile_pool(name="stg", bufs=3))
    ps = ctx.enter_context(tc.tile_pool(name="ps", bufs=1, space="PSUM"))

    # ---- constants / weights in SBUF ----
    wq_sb = const.tile([P, NDT, FPC], F16)
    nc.sync.dma_start(wq_sb[:], wqT[:])
    wk_sb = const.tile([P, NDT, FPC], F16)
    nc.sync.dma_start(wk_sb[:], wkT[:])
    wv_sb = const.tile([P, NDT, FPC], F16)
    nc.scalar.dma_start(wv_sb[:], wvT[:])
    # wo is first needed at the first folded phase 4; its DMA is issued
    # after phase 1 (below) so it doesn't delay the x stream on either ring
    wo_sb = const.tile([P, FPC // P, D], F16)
    ones_bc = const.tile([P, DH], F16)
    nc.vector.memset(ones_bc[:], 1.0 / DH)

    # ---- persistent activations ----
    q_sb = [sb.tile([P, S], F16, name=f"q_sb{i}") for i in range(2)]
    k_sb = [sb.tile([P, S], F16, name=f"k_sb{i}") for i in range(2)]
    v_sb = [sb.tile([P, 2, 192], F16, name=f"v_sb{i}") for i in range(NKT)]
    ctx_sb = [sb.tile([P, S], F16, name=f"ctx_sb{i}") for i in range(2)]

    # ---- phase 1: projections (+ mask prefetch/expansion on idle DVE) ----
    # x chunks spread over all three DMA paths (sync/scalar HWDGE + the
    # otherwise-idle gpsimd SWDGE ring) so the input head isn't serialized
    # on one ring if the fabric throttles per-ring
    x_eng = (nc.sync, nc.scalar, nc.gpsimd, nc.sync)
    x_eng2 = (nc.scalar, nc.gpsimd, nc.sync, nc.gpsimd)
    keeps = []
    for sc in range(NQC):
        xt_sc = xtp.tile([P, NDT, SC], F16, tag="xt", name=f"xt_{sc}")
        # each chunk lands as two half-chunks on different rings: the
        # dt 0-3 projection matmuls start after 0.5MB instead of 1MB
        h = NDT // 2
        x_eng[sc].dma_start(xt_sc[:, 0:h, :], xT[sc, :, 0:h, :])
        x_eng2[sc].dma_start(xt_sc[:, h:NDT, :], xT[sc, :, h:NDT, :])
        # mask qc=sc ships bit-packed (16 q per int16 word, bit-plane-major)
        # and expands here, in phase 1, where DVE has slack — expanding at
        # the qc boundary inside the attention loop stalls ACT ~5us per qc.
        kb_sb = keepp.tile([P, NKT, W16], I16, tag="kbits", name=f"kb_{sc}")
        # kb0 rides the gpsimd ring's free queue head: behind weights+x on
        # the sync ring its expansion would gate the first mask-AND
        (nc.gpsimd if sc % 2 == 0 else nc.sync).dma_start(kb_sb[:], keepT[sc])
        keep_sb = keepp.tile([P, NKT, SC], I16, tag="keep", name=f"keep_{sc}")
        for b in range(16):
            nc.vector.tensor_scalar(
                keep_sb[:, :, b * W16:(b + 1) * W16], kb_sb[:],
                b, 1, LSR, op1=AND)
        nc.vector.tensor_scalar(keep_sb[:], keep_sb[:], -1, None, MULT)
        keeps.append(keep_sb)
        if VARIANT == "dma":
            continue
        for pair in range(2):
            for wi, (w_sb, dst) in enumerate(((wq_sb, q_sb), (wk_sb, k_sb))):
                mm = ps.tile([P, SC], F32, tag=("ctxX", "ctxY")[(2 * pair + wi) % 2],
                             bufs=2, name=f"qk_{sc}_{pair}_{wi}")
                for dt in range(NDT):
                    nc.tensor.matmul(
                        mm[:],
                        w_sb[:, dt, pair * P:(pair + 1) * P],
                        xt_sc[:, dt, :],
                        start=(dt == 0),
                        stop=(dt == NDT - 1),
                    )
                # ACT is idle during phase 1; use it for q/k evictions
                nc.scalar.copy(dst[pair][:, sc * SC:(sc + 1) * SC], mm[:])
        for ssub in range(SC // P):
            kt = sc * (SC // P) + ssub
            vm = ps.tile([P, FPC], F32, tag=("ctxX", "ctxY")[kt % 2], bufs=2,
                         name=f"v_{kt}")
            for dt in range(NDT):
                nc.tensor.matmul(
                    vm[:],
                    xt_sc[:, dt, ssub * P:(ssub + 1) * P],
                    wv_sb[:, dt, :],
                    start=(dt == 0),
                    stop=(dt == NDT - 1),
                )
            for pr in range(2):
                nc.vector.tensor_copy(v_sb[kt][:, pr, 0:DH],
                                      vm[:, pr * P:pr * P + DH])
                nc.vector.tensor_copy(v_sb[kt][:, pr, 2 * DH:3 * DH],
                                      vm[:, pr * P + DH:(pr + 1) * P])
            nc.vector.memset(v_sb[kt][:, :, DH:2 * DH], 1.0)

    nc.scalar.dma_start(wo_sb[:], woT[:])

    # ---- phases 2+3+4: attention + interleaved output projection ----
    # phase 4 for chunk qc-1 is emitted one ft-tile at a time inside the
    # kt loop of chunk qc's pair 0: the PE has ~400ns of slack per
    # ACT-bound iteration, so the output projection rides along instead of
    # forming a serial PE block that stalls the next chunk's scores.
    def emit_p4_ft(pqc, ft, st):
        pqsl = slice(pqc * SC, (pqc + 1) * SC)
        om = ps.tile([P, SC], F32, tag=("ctxX", "ctxY")[ft % 2], bufs=2,
                     name=f"o_{pqc}_{ft}")
        for ph in range(FPC // P):
            nc.tensor.matmul(
                om[:],
                wo_sb[:, ph, ft * P:(ft + 1) * P],
                ctx_sb[ph][:, pqsl],
                start=(ph == 0),
                stop=(ph == FPC // P - 1),
            )
        nc.vector.tensor_copy(st[:, ft, :], om[:])
        if ft % 2 == 1:
            # alternate output pairs across the scalar and gpsimd rings to
            # balance ring load (scalar also carries wv/x1/wo inputs)
            eng = nc.scalar if (ft // 2) % 2 == 0 else nc.gpsimd
            eng.dma_start(outT[pqc, :, ft - 1:ft + 1, :],
                          st[:, ft - 1:ft + 1, :])

    for qc in range(NQC):
        keep_sb = keeps[qc]
        if VARIANT in ("dma", "noattn"):
            continue
        qsl = slice(qc * SC, (qc + 1) * SC)
        st_prev = None
        if qc > 0:
            st_prev = stg.tile([P, NDT, SC], F16, tag="stage", bufs=2,
                               name=f"st_{qc - 1}")
        for pair in range(2):
            # bank Y: ctx_a on [0:64], denom_a (x64 replicated) on [64:128]
            # bank X: denom_b (x64) on [0:64], ctx_b on [64:128]
            ctx_y = ps.tile([P, SC], F32, tag="ctxY", bufs=2, name=f"ctxY_{qc}_{pair}")
            ctx_x = ps.tile([P, SC], F32, tag="ctxX", bufs=2, name=f"ctxX_{qc}_{pair}")

            for kt in range(NKT):
                # both heads' scores in one 2-bank PSUM tile -> one fused exp
                scb = ps.tile([P, 2, SC], F32, tag="scAB", bufs=2,
                              name=f"sc_{qc}_{pair}_{kt}")
                ksl = slice(kt * P, (kt + 1) * P)
                nc.tensor.matmul(
                    scb[:, 0, :],
                    k_sb[pair][0:DH, ksl],
                    q_sb[pair][0:DH, qsl],
                    start=True, stop=True,
                )
                nc.tensor.matmul(
                    scb[:, 1, :],
                    k_sb[pair][DH:P, ksl],
                    q_sb[pair][DH:P, qsl],
                    start=True, stop=True,
                    tile_position=(64, 0),
                )
                w = wp.tile([P, 2, SC], F16, tag="w", name=f"w_{qc}_{pair}_{kt}")
                nc.scalar.activation(w[:].rearrange("p h q -> p (h q)"),
                                     scb[:].rearrange("p h q -> p (h q)"), EXP)
                kb = keep_sb[:, kt, :][:, None, :].to_broadcast((P, 2, SC))
                nc.vector.tensor_tensor(w[:].bitcast(I16), w[:].bitcast(I16),
                                        kb, AND)
                vt = v_sb[kt]
                first, last = kt == 0, kt == NKT - 1
                nc.tensor.matmul(
                    ctx_y[:], vt[:, pair, 0:2 * DH], w[:, 0, :],
                    start=first, stop=last,
                )
                nc.tensor.matmul(
                    ctx_x[:], vt[:, pair, DH:3 * DH], w[:, 1, :],
                    start=first, stop=last,
                )
                if st_prev is not None and kt % 4 == 3:
                    # spread the previous chunk's 8 output-projection tiles
                    # across both pairs (one per 4 kt) so the added PE work
                    # (+2 LDW + 2 MM each) stays under the ACT-bound slack
                    emit_p4_ft(qc - 1, 4 * pair + kt // 4, st_prev)
            recip = stg.tile([P, SC], F16, tag="recip", name=f"recip_{qc}_{pair}")
            with nc.allow_low_precision(reason="softmax denom reciprocal in fp16"):
                nc.vector.reciprocal(recip[0:DH, :], ctx_x[0:DH, :])
                nc.vector.reciprocal(recip[DH:P, :], ctx_y[DH:P, :])
            # broadcast the reciprocals back into the now-dead denominator
            # halves of the ctx banks (keeps the scores tag free: allocating
            # bc from scAB stalls qc+1's scores ~5us at every qc boundary),
            # then normalize with direct PSUMxPSUM multiplies.
            nc.tensor.matmul(
                ctx_x[0:DH, :], ones_bc[DH:P, 0:DH], recip[DH:P, :],
                start=True, stop=True, tile_position=(64, 0),
            )
            nc.tensor.matmul(
                ctx_y[DH:P, :], ones_bc[0:DH, 0:DH], recip[0:DH, :],
                start=True, stop=True, tile_position=(0, 64),
            )
            # DVE may read only one PSUM operand per instruction: stage the
            # broadcast reciprocals to SBUF before the normalize multiplies
            rcp2 = stg.tile([P, SC], F16, tag="recip2", name=f"rcp2_{qc}_{pair}")
            nc.vector.tensor_copy(rcp2[0:DH, :], ctx_x[0:DH, :])
            nc.vector.tensor_copy(rcp2[DH:P, :], ctx_y[DH:P, :])
            nc.vector.tensor_tensor(
                ctx_sb[pair][0:DH, qsl], ctx_y[0:DH, :], rcp2[0:DH, :], MULT)
            nc.vector.tensor_tensor(
                ctx_sb[pair][DH:P, qsl], ctx_x[DH:P, :], rcp2[DH:P, :], MULT)

    if VARIANT == "full":
        # trailing phase 4 for the last chunk
        st = stg.tile([P, NDT, SC], F16, tag="stage", bufs=2, name="st_last")
        for ft in range(NDT):
            emit_p4_ft(NQC - 1, ft, st)

    if VARIANT in ("dma", "noattn"):
        # still produce the output DMAs so the byte count matches
        for qc in range(NQC):
            st = stg.tile([P, NDT, SC], F16, tag="stage", bufs=2, name=f"zst_{qc}")
            nc.vector.memset(st[:], 0.0)
            nc.sync.dma_start(outT[qc], st[:])


def build():
    nc = bacc.Bacc("TRN2", target_bir_lowering=False, debug=False, num_devices=NCORES)
    # all inputs pre-tiled on the host so every DMA line is contiguous
    xT = nc.dram_tensor("xT", [NQC, P, NDT, SC], F16, kind="ExternalInput").ap()
    wqT = nc.dram_tensor("wqT", [P, NDT, FPC], F16, kind="ExternalInput").ap()
    wkT = nc.dram_tensor("wkT", [P, NDT, FPC], F16, kind="ExternalInput").ap()
    wvT = nc.dram_tensor("wvT", [P, NDT, FPC], F16, kind="ExternalInput").ap()
    woT = nc.dram_tensor("woT", [P, FPC // P, D], F16, kind="ExternalInput").ap()
    keepT = nc.dram_tensor("keepT", [NQC, P, NKT, W16], I16, kind="ExternalInput").ap()
    outT = nc.dram_tensor("outT", [NQC, P, NDT, SC], F16, kind="ExternalOutput").ap()
    with tile.TileContext(nc) as tc, ExitStack() as ctx:
        _emit(ctx, tc, xT, wqT, wkT, wvT, woT, keepT, outT)
    nc.compile()
    return nc


def make_in_maps(query, mask, Wq, Wk, Wv, Wo):
    scale = 1.0 / math.sqrt(DH)
    in_maps = []
    for b in range(B):
        # xT tiled: [NQC, P, NDT, SC]; element (sc, p, dt, s) = x[sc*SC+s, dt*P+p]
        xt = query[b].astype(np.float16).T.reshape(NDT, P, NQC, SC)
        xT = np.ascontiguousarray(xt.transpose(2, 1, 0, 3))
        # keep bit-packed: [NQC, P, NKT, W16] int16; bit b of word (qc,p,kt,w)
        # = keep[kt*P+p, qc*SC + b*W16 + w]
        kp = (~mask[b]).T.astype(np.uint16).reshape(NKT, P, NQC, 16, W16)
        bits = (kp << np.arange(16, dtype=np.uint16)[None, None, None, :, None])
        bits = bits.sum(3, dtype=np.uint16)
        keepT = np.ascontiguousarray(
            bits.transpose(2, 1, 0, 3)).view(np.int16)
        for g in range(GROUPS):
            f0 = g * FPC
            def pack_w(wT):  # [D, FPC] -> [P, NDT, FPC]
                return np.ascontiguousarray(
                    wT.reshape(NDT, P, FPC).transpose(1, 0, 2))
            in_maps.append({
                "xT": xT,
                "wqT": pack_w((Wq[f0:f0 + FPC, :] * scale).T.astype(np.float16)),
                "wkT": pack_w(Wk[f0:f0 + FPC, :].T.astype(np.float16)),
                "wvT": pack_w(Wv[f0:f0 + FPC, :].T.astype(np.float16)),
                "woT": np.ascontiguousarray(
                    Wo[:, f0:f0 + FPC].T.astype(np.float16)
                    .reshape(FPC // P, P, D).transpose(1, 0, 2)),
                "keepT": keepT,
            })
    return in_maps


_NC_CACHE = {}


def _get_nc():
    if "nc" not in _NC_CACHE:
        _NC_CACHE["nc"] = build()
    return _NC_CACHE["nc"]


def gather(results, bo):
    out = np.empty((B, S, D), dtype=np.float32)
    for b in range(B):
        acc = results[b * GROUPS]["outT"].astype(np.float32)
        for g in range(1, GROUPS):
            acc = acc + results[b * GROUPS + g]["outT"].astype(np.float32)
        # outT [NQC, P, NDT, SC]: (qc, p, ft, q) = out_part[ft*P+p, qc*SC+q]
        full = acc.transpose(2, 1, 0, 3).reshape(D, S)
        out[b] = full.T + bo.astype(np.float32)
    return out


def kernel(query, mask, Wq, Wk, Wv, Wo, bo, **kwargs):
    nc = _get_nc()
    in_maps = make_in_maps(np.asarray(query), np.asarray(mask), np.asarray(Wq),
                           np.asarray(Wk), np.asarray(Wv), np.asarray(Wo))
    res = run_bass_kernel_spmd(nc, in_maps, list(range(NCORES)))
    return gather(res.results, np.asarray(bo))
